# revision 5
# baseline (speedup 1.0000x reference)
"""Chamfer distance kernel for 8 Trainium2 NeuronCores (Bass/Tile), v2.

Problem: xyz1, xyz2: (4, 8192, 3) fp32. Outputs dist1, dist2: (4, 8192) fp32,
the row-wise / column-wise minima of the pairwise squared-distance matrix
d[n,m] = max(||x_n||^2 + ||y_m||^2 - 2 x_n.y_m, 0), per batch.

Sharding: core c handles batch c//2 and half of the N rows (c%2). Each core
computes dist1 for its 4096 rows exactly, and a dist2 partial (min over its
4096 rows) for all 8192 columns; the host min-combines the two partials.

v2 design (vs the v1 both-orientations kernel at ~554us): the distance matrix
is computed ONCE per core (TensorE, K=24 bf16-augmented matmuls, fp32 PSUM),
ScalarE evacuates each PSUM group as relu'd BF16 into SBUF (1x, ~2.05us per
[128,2048]), and ALL min work runs on the DVE as bf16 SBUF tensor_tensor ops
which hit the 2x_1P perf mode (measured 2194ns for a [128,4096] TT min =
(58+2048)cyc @0.96GHz; reduce-class ops are stuck at 1x):

  - dist2: running elementwise-min accumulator colacc[128, 8192] over the 32
    row-tiles (2 TT ops per row-tile). The final cross-partition min runs as
    64 TensorE transposes of colacc blocks into bf16 bitcast views of the
    PSUM ring + 8 DVE tensor_reduce ops.
  - dist1: per row-tile a 2-level TT tree (8192 -> 2048) whose output lands
    in a [128, 8, 2048] batch buffer; every 8 row-tiles, 4 more batched tree
    levels (3D APs) + one small 1x reduce produce 8 row-min columns at once.

PSUM: one [128, 8, 512] f32 tile used as a ring; 4 matmuls (FD=512) fill a
[128, 4, 512] half, ScalarE reads it as one 2048-wide ACTIVATE. Subtile deps
give WAR/RAW ordering. The d2 transposes at the tail of the body overlap the
next repeat iteration's head, so the steady-state cost is engine-bound.
"""

from contextlib import ExitStack

import numpy as np
import ml_dtypes

B, N, M = 4, 8192, 8192
NCORES = 8
NLOC = N // 2          # rows of xyz1 per core
P = 128                # partitions
FD = 512               # matmul free dim (one PSUM bank of fp32)
KAUG = 24

_BF16 = ml_dtypes.bfloat16


def _decomp3(v):
    """fp32/fp64 array -> three bf16 planes summing to v (residual ~2^-27)."""
    v = v.astype(np.float32)
    h = v.astype(_BF16)
    r = v - h.astype(np.float32)
    m = r.astype(_BF16)
    r2 = r - m.astype(np.float32)
    l = r2.astype(_BF16)
    return h, m, l


def _build_aug(x, y):
    """x: [Nl,3] fp32, y: [Mm,3] fp32 -> (xa [KAUG,Nl] bf16, ya [KAUG,Mm] bf16).

    d[n,m] = sum_k xa[k,n]*ya[k,m] up to bf16x3 residuals. Slot order puts the
    large mutually-cancelling terms first so fp32 PSUM accumulation stays
    accurate near d ~ 0.
    """
    nl, mm = x.shape[0], y.shape[0]
    nx = (x.astype(np.float64) ** 2).sum(axis=1)
    ny = (y.astype(np.float64) ** 2).sum(axis=1)
    xh, xm, xl = _decomp3(x)
    y2 = (-2.0 * y.astype(np.float64)).astype(np.float32)
    yh, ym, yl = _decomp3(y2)
    nxh, nxm, nxl = _decomp3(nx)
    nyh, nym, nyl = _decomp3(ny)

    one_n = np.ones(nl, dtype=_BF16)
    one_m = np.ones(mm, dtype=_BF16)

    xa = np.empty((KAUG, nl), dtype=_BF16)
    ya = np.empty((KAUG, mm), dtype=_BF16)
    k = 0

    def slot(xv, yv):
        nonlocal k
        xa[k] = xv
        ya[k] = yv
        k += 1

    slot(nxh, one_m)
    slot(one_n, nyh)
    for c in range(3):
        slot(xh[:, c], yh[:, c])
    slot(nxm, one_m)
    slot(one_n, nym)
    for c in range(3):
        slot(xh[:, c], ym[:, c])
    for c in range(3):
        slot(xm[:, c], yh[:, c])
    slot(nxl, one_m)
    slot(one_n, nyl)
    for c in range(3):
        slot(xh[:, c], yl[:, c])
    for c in range(3):
        slot(xm[:, c], ym[:, c])
    for c in range(3):
        slot(xl[:, c], yh[:, c])
    assert k == KAUG
    return xa, ya


def build_bass(nloc=NLOC, m_total=M, repeat=1, s_bufs=3, s_pad=128, t_pad=64):
    """Build + compile the per-core Bass program.

    repeat>1 wraps the main compute in a dynamic loop executing it `repeat`
    times — used only to measure per-iteration HW time above the PJRT
    dispatch noise floor. The loop body is self-contained (accumulators are
    re-initialized inside), so outputs are repeat-invariant.

    s_pad/t_pad pad the bf16 SBUF tiles (elements per buf) to shift SBUF
    bank phase between pool slots (ScalarE-write vs DVE-read conflicts).
    """
    import concourse.bacc as bacc
    import concourse.tile as tile
    import concourse.mybir as mybir

    f32 = mybir.dt.float32
    bf16 = mybir.dt.bfloat16
    Alu = mybir.AluOpType
    Act = mybir.ActivationFunctionType
    X = mybir.AxisListType.X

    ntile = nloc // P             # 32 row-tiles
    ngrp = m_total // (4 * FD)    # 4 ScalarE groups of 2048 per row-tile
    nbatch = 8                    # row-tiles per dist1 batch phase
    nph = ntile // nbatch         # 4 batch phases
    nblk = m_total // P           # 64 transpose blocks for dist2

    nc = bacc.Bacc("TRN2", target_bir_lowering=False, debug=False)
    xa_d = nc.dram_tensor("xa", [KAUG, nloc], bf16, kind="ExternalInput")
    ya_d = nc.dram_tensor("ya", [KAUG, m_total], bf16, kind="ExternalInput")
    eye_d = nc.dram_tensor("eye", [P, P], bf16, kind="ExternalInput")
    d1_d = nc.dram_tensor("d1", [P, ntile], f32, kind="ExternalOutput")
    d2_d = nc.dram_tensor("d2", [P, nblk], f32, kind="ExternalOutput")

    with tile.TileContext(nc) as tc, ExitStack() as ctx:
        singles = ctx.enter_context(tc.tile_pool(name="singles", bufs=1))
        psum = ctx.enter_context(tc.tile_pool(name="psum", bufs=1, space="PSUM"))

        # chunked loads so the first matmuls start before the full tensors land
        xa = singles.tile([KAUG, nloc], bf16)
        for i in range(4):
            sl = slice(i * nloc // 4, (i + 1) * nloc // 4)
            nc.sync.dma_start(out=xa[:, sl], in_=xa_d.ap()[:, sl])
        ya = singles.tile([KAUG, m_total], bf16)
        for i in range(4):
            sl = slice(i * m_total // 4, (i + 1) * m_total // 4)
            nc.sync.dma_start(out=ya[:, sl], in_=ya_d.ap()[:, sl])
        eye = singles.tile([P, P], bf16)
        nc.sync.dma_start(out=eye, in_=eye_d.ap())

        spool = ctx.enter_context(tc.tile_pool(name="spool", bufs=s_bufs))

        colacc = singles.tile([P, m_total], bf16)
        rp = singles.tile([P, nbatch, 4096], bf16)
        c1 = singles.tile([P, ntile], f32)
        c2 = singles.tile([P, nblk], f32)
        d1t = singles.tile([P, ntile], f32)
        d2t = singles.tile([P, nblk], f32)

        # PSUM ring: [128, 8, 512] f32 = all 8 banks. Halves (4 slices =
        # 2048 f32) alternate between PE writes and one ScalarE ACTIVATE.
        pring = psum.tile([P, 8, FD], f32, name="pring")
        pring_bf = pring.bitcast(bf16)  # [128, 8, 1024] for d2 transposes

        def body():
            nc.gpsimd.memset(colacc, 3.0e38)

            for it in range(ntile):
                s = spool.tile(
                    [P, m_total], bf16, name="s", tag="s",
                    padded_shape=[P, m_total + s_pad],
                )
                for g in range(ngrp):
                    h = g % 2
                    for j in range(4):
                        nc.tensor.matmul(
                            pring[:, 4 * h + j, :],
                            xa[:, it * P : (it + 1) * P],
                            ya[:, (g * 4 + j) * FD : (g * 4 + j + 1) * FD],
                            start=True,
                            stop=True,
                        )
                    nc.scalar.activation(
                        out=s[:, g * 2048 : (g + 1) * 2048],
                        in_=pring[:, 4 * h : 4 * h + 4, :].rearrange(
                            "p a b -> p (a b)"
                        ),
                        func=Act.Relu,
                    )
                # dist2 column accumulation: 2x 4096-wide TT min
                for half in range(2):
                    sl = slice(half * 4096, (half + 1) * 4096)
                    nc.vector.tensor_tensor(
                        out=colacc[:, sl], in0=colacc[:, sl], in1=s[:, sl],
                        op=Alu.min,
                    )
                # dist1 per-tile tree level 1: 8192 -> 4096 into batch buf
                nc.vector.tensor_tensor(
                    out=rp[:, it % nbatch, :],
                    in0=s[:, :4096],
                    in1=s[:, 4096:],
                    op=Alu.min,
                )
                # batched deep levels every nbatch row-tiles
                if it % nbatch == nbatch - 1:
                    ph = it // nbatch
                    w = 4096
                    while w > 64:
                        nc.vector.tensor_tensor(
                            out=rp[:, :, : w // 2],
                            in0=rp[:, :, : w // 2],
                            in1=rp[:, :, w // 2 : w],
                            op=Alu.min,
                        )
                        w //= 2
                    nc.vector.tensor_reduce(
                        out=c1[:, ph * nbatch : (ph + 1) * nbatch],
                        in_=rp[:, :, :64],
                        axis=X,
                        op=Alu.min,
                    )

            # dist2 finals: transpose colacc blocks into PSUM (bf16 views of
            # the ring slices), reduce each slice's 8 blocks on the DVE.
            for sl8 in range(8):
                for k in range(8):
                    blk = sl8 * 8 + k
                    nc.tensor.transpose(
                        out=pring_bf[:, sl8, k * P : (k + 1) * P],
                        in_=colacc[:, blk * P : (blk + 1) * P],
                        identity=eye,
                    )
                nc.vector.tensor_reduce(
                    out=c2[:, sl8 * 8 : (sl8 + 1) * 8],
                    in_=pring_bf[:, sl8, :].rearrange("p (b f) -> p b f", f=P),
                    axis=X,
                    op=Alu.min,
                )

            nc.vector.tensor_scalar_max(out=d1t, in0=c1, scalar1=0.0)
            nc.vector.tensor_scalar_max(out=d2t, in0=c2, scalar1=0.0)

        if repeat == 1:
            body()
        else:
            with tc.For_i(0, repeat, 1):
                body()

        nc.sync.dma_start(out=d1_d.ap(), in_=d1t)
        nc.sync.dma_start(out=d2_d.ap(), in_=d2t)

    nc.compile()
    return nc


_CACHED_NC = None


def _get_nc():
    global _CACHED_NC
    if _CACHED_NC is None:
        _CACHED_NC = build_bass()
    return _CACHED_NC


_EYE = np.eye(P, dtype=_BF16)


def _make_in_maps(xyz1, xyz2):
    xyz1 = np.asarray(xyz1, dtype=np.float32)
    xyz2 = np.asarray(xyz2, dtype=np.float32)
    in_maps = []
    for c in range(NCORES):
        b, h = divmod(c, 2)
        x = xyz1[b, h * NLOC : (h + 1) * NLOC]
        y = xyz2[b]
        xa, ya = _build_aug(x, y)
        in_maps.append({"xa": xa, "ya": ya, "eye": _EYE})
    return in_maps


def _unshard(results):
    dist1 = np.empty((B, N), np.float32)
    dist2 = np.empty((B, M), np.float32)
    for c in range(NCORES):
        b, h = divmod(c, 2)
        dist1[b, h * NLOC : (h + 1) * NLOC] = np.asarray(results[c]["d1"]).T.ravel()
        d2p = np.asarray(results[c]["d2"]).T.ravel()
        if h == 0:
            dist2[b] = d2p
        else:
            np.minimum(dist2[b], d2p, out=dist2[b])
    return dist1, dist2


def kernel(xyz1, xyz2):
    from concourse.bass_utils import run_bass_kernel_spmd

    nc = _get_nc()
    in_maps = _make_in_maps(xyz1, xyz2)
    res = run_bass_kernel_spmd(nc, in_maps, core_ids=list(range(NCORES)))
    return _unshard(res.results)


# revision 7
# speedup vs baseline: 1.0031x; 1.0031x over previous
"""Chamfer distance kernel for 8 Trainium2 NeuronCores (Bass/Tile), v2.

Problem: xyz1, xyz2: (4, 8192, 3) fp32. Outputs dist1, dist2: (4, 8192) fp32,
the row-wise / column-wise minima of the pairwise squared-distance matrix
d[n,m] = max(||x_n||^2 + ||y_m||^2 - 2 x_n.y_m, 0), per batch.

Sharding: core c handles batch c//2 and half of the N rows (c%2). Each core
computes dist1 for its 4096 rows exactly, and a dist2 partial (min over its
4096 rows) for all 8192 columns; the host min-combines the two partials.

v2 design (vs the v1 both-orientations kernel at ~554us): the distance matrix
is computed ONCE per core (TensorE, K=24 bf16-augmented matmuls, fp32 PSUM),
ScalarE evacuates each PSUM group as relu'd BF16 into SBUF (1x, ~2.05us per
[128,2048]), and ALL min work runs on the DVE as bf16 SBUF tensor_tensor ops
which hit the 2x_1P perf mode (measured 2194ns for a [128,4096] TT min =
(58+2048)cyc @0.96GHz; reduce-class ops are stuck at 1x):

  - dist2: running elementwise-min accumulator colacc[128, 8192] over the 32
    row-tiles (2 TT ops per row-tile). The final cross-partition min runs as
    64 TensorE transposes of colacc blocks into bf16 bitcast views of the
    PSUM ring + 8 DVE tensor_reduce ops.
  - dist1: per row-tile a 2-level TT tree (8192 -> 2048) whose output lands
    in a [128, 8, 2048] batch buffer; every 8 row-tiles, 4 more batched tree
    levels (3D APs) + one small 1x reduce produce 8 row-min columns at once.

PSUM: one [128, 8, 512] f32 tile used as a ring; 4 matmuls (FD=512) fill a
[128, 4, 512] half, ScalarE reads it as one 2048-wide ACTIVATE. Subtile deps
give WAR/RAW ordering. The d2 transposes at the tail of the body overlap the
next repeat iteration's head, so the steady-state cost is engine-bound.
"""

from contextlib import ExitStack

import numpy as np
import ml_dtypes

B, N, M = 4, 8192, 8192
NCORES = 8
NLOC = N // 2          # rows of xyz1 per core
P = 128                # partitions
FD = 512               # matmul free dim (one PSUM bank of fp32)
KAUG = 24

_BF16 = ml_dtypes.bfloat16


def _decomp3(v):
    """fp32/fp64 array -> three bf16 planes summing to v (residual ~2^-27)."""
    v = v.astype(np.float32)
    h = v.astype(_BF16)
    r = v - h.astype(np.float32)
    m = r.astype(_BF16)
    r2 = r - m.astype(np.float32)
    l = r2.astype(_BF16)
    return h, m, l


def _build_aug(x, y):
    """x: [Nl,3] fp32, y: [Mm,3] fp32 -> (xa [KAUG,Nl] bf16, ya [KAUG,Mm] bf16).

    d[n,m] = sum_k xa[k,n]*ya[k,m] up to bf16x3 residuals. Slot order puts the
    large mutually-cancelling terms first so fp32 PSUM accumulation stays
    accurate near d ~ 0.
    """
    nl, mm = x.shape[0], y.shape[0]
    nx = (x.astype(np.float64) ** 2).sum(axis=1)
    ny = (y.astype(np.float64) ** 2).sum(axis=1)
    xh, xm, xl = _decomp3(x)
    y2 = (-2.0 * y.astype(np.float64)).astype(np.float32)
    yh, ym, yl = _decomp3(y2)
    nxh, nxm, nxl = _decomp3(nx)
    nyh, nym, nyl = _decomp3(ny)

    one_n = np.ones(nl, dtype=_BF16)
    one_m = np.ones(mm, dtype=_BF16)

    xa = np.empty((KAUG, nl), dtype=_BF16)
    ya = np.empty((KAUG, mm), dtype=_BF16)
    k = 0

    def slot(xv, yv):
        nonlocal k
        xa[k] = xv
        ya[k] = yv
        k += 1

    slot(nxh, one_m)
    slot(one_n, nyh)
    for c in range(3):
        slot(xh[:, c], yh[:, c])
    slot(nxm, one_m)
    slot(one_n, nym)
    for c in range(3):
        slot(xh[:, c], ym[:, c])
    for c in range(3):
        slot(xm[:, c], yh[:, c])
    slot(nxl, one_m)
    slot(one_n, nyl)
    for c in range(3):
        slot(xh[:, c], yl[:, c])
    for c in range(3):
        slot(xm[:, c], ym[:, c])
    for c in range(3):
        slot(xl[:, c], yh[:, c])
    assert k == KAUG
    return xa, ya


def build_bass(
    nloc=NLOC, m_total=M, repeat=1, s_bufs=3, s_pad=128, t_pad=64,
    row_mode="t1", rp_pad=0, nbatch=8,
):
    """Build + compile the per-core Bass program.

    repeat>1 wraps the main compute in a dynamic loop executing it `repeat`
    times — used only to measure per-iteration HW time above the PJRT
    dispatch noise floor. The loop body is self-contained (accumulators are
    re-initialized inside), so outputs are repeat-invariant.

    s_pad/t_pad pad the bf16 SBUF tiles (elements per buf) to shift SBUF
    bank phase between pool slots (ScalarE-write vs DVE-read conflicts).
    """
    import concourse.bacc as bacc
    import concourse.tile as tile
    import concourse.mybir as mybir

    f32 = mybir.dt.float32
    bf16 = mybir.dt.bfloat16
    Alu = mybir.AluOpType
    Act = mybir.ActivationFunctionType
    X = mybir.AxisListType.X

    ntile = nloc // P             # 32 row-tiles
    ngrp = m_total // (4 * FD)    # 4 ScalarE groups of 2048 per row-tile
    nblk = m_total // P           # 64 transpose blocks for dist2

    nc = bacc.Bacc("TRN2", target_bir_lowering=False, debug=False)
    xa_d = nc.dram_tensor("xa", [KAUG, nloc], bf16, kind="ExternalInput")
    ya_d = nc.dram_tensor("ya", [KAUG, m_total], bf16, kind="ExternalInput")
    eye_d = nc.dram_tensor("eye", [P, P], bf16, kind="ExternalInput")
    d1_d = nc.dram_tensor("d1", [P, ntile], f32, kind="ExternalOutput")
    d2_d = nc.dram_tensor("d2", [P, nblk], f32, kind="ExternalOutput")

    with tile.TileContext(nc) as tc, ExitStack() as ctx:
        singles = ctx.enter_context(tc.tile_pool(name="singles", bufs=1))
        psum = ctx.enter_context(tc.tile_pool(name="psum", bufs=1, space="PSUM"))

        # chunked loads so the first matmuls start before the full tensors land
        xa = singles.tile([KAUG, nloc], bf16)
        for i in range(4):
            sl = slice(i * nloc // 4, (i + 1) * nloc // 4)
            nc.sync.dma_start(out=xa[:, sl], in_=xa_d.ap()[:, sl])
        ya = singles.tile([KAUG, m_total], bf16)
        for i in range(4):
            sl = slice(i * m_total // 4, (i + 1) * m_total // 4)
            nc.sync.dma_start(out=ya[:, sl], in_=ya_d.ap()[:, sl])
        eye = singles.tile([P, P], bf16)
        nc.sync.dma_start(out=eye, in_=eye_d.ap())

        spool = ctx.enter_context(tc.tile_pool(name="spool", bufs=s_bufs))
        if row_mode == "t1":
            tpool = ctx.enter_context(tc.tile_pool(name="tpool", bufs=2))

        colacc = singles.tile([P, m_total], bf16)
        rp_w = 2048 if row_mode == "t1" else 4096
        rp = singles.tile(
            [P, nbatch, rp_w], bf16,
            padded_shape=[P, nbatch, rp_w + rp_pad] if rp_pad else None,
        )
        c1 = singles.tile([P, ntile], f32)
        c2 = singles.tile([P, nblk], f32)
        d1t = singles.tile([P, ntile], f32)
        d2t = singles.tile([P, nblk], f32)

        # PSUM ring: [128, 8, 512] f32 = all 8 banks. Halves (4 slices =
        # 2048 f32) alternate between PE writes and one ScalarE ACTIVATE.
        pring = psum.tile([P, 8, FD], f32, name="pring")
        pring_bf = pring.bitcast(bf16)  # [128, 8, 1024] for d2 transposes

        def body():
            nc.gpsimd.memset(colacc, 3.0e38)

            for it in range(ntile):
                s = spool.tile(
                    [P, m_total], bf16, name="s", tag="s",
                    padded_shape=[P, m_total + s_pad],
                )
                for g in range(ngrp):
                    h = g % 2
                    for j in range(4):
                        nc.tensor.matmul(
                            pring[:, 4 * h + j, :],
                            xa[:, it * P : (it + 1) * P],
                            ya[:, (g * 4 + j) * FD : (g * 4 + j + 1) * FD],
                            start=True,
                            stop=True,
                        )
                    nc.scalar.activation(
                        out=s[:, g * 2048 : (g + 1) * 2048],
                        in_=pring[:, 4 * h : 4 * h + 4, :].rearrange(
                            "p a b -> p (a b)"
                        ),
                        func=Act.Relu,
                    )
                # dist2 column accumulation: 2x 4096-wide TT min
                for half in range(2):
                    sl = slice(half * 4096, (half + 1) * 4096)
                    nc.vector.tensor_tensor(
                        out=colacc[:, sl], in0=colacc[:, sl], in1=s[:, sl],
                        op=Alu.min,
                    )
                # dist1 per-tile tree into the batch buffer
                if row_mode == "t1":
                    t1 = tpool.tile(
                        [P, 4096], bf16, name="t1", tag="t1",
                        padded_shape=[P, 4096 + t_pad],
                    )
                    nc.vector.tensor_tensor(
                        out=t1, in0=s[:, :4096], in1=s[:, 4096:], op=Alu.min
                    )
                    nc.vector.tensor_tensor(
                        out=rp[:, it % nbatch, :],
                        in0=t1[:, :2048],
                        in1=t1[:, 2048:],
                        op=Alu.min,
                    )
                else:
                    nc.vector.tensor_tensor(
                        out=rp[:, it % nbatch, :],
                        in0=s[:, :4096],
                        in1=s[:, 4096:],
                        op=Alu.min,
                    )
                # batched deep levels every nbatch row-tiles
                if it % nbatch == nbatch - 1:
                    ph = it // nbatch
                    w = rp_w
                    while w > 64:
                        nc.vector.tensor_tensor(
                            out=rp[:, :, : w // 2],
                            in0=rp[:, :, : w // 2],
                            in1=rp[:, :, w // 2 : w],
                            op=Alu.min,
                        )
                        w //= 2
                    nc.vector.tensor_reduce(
                        out=c1[:, ph * nbatch : (ph + 1) * nbatch],
                        in_=rp[:, :, :64],
                        axis=X,
                        op=Alu.min,
                    )

            # dist2 finals: transpose colacc blocks into PSUM (bf16 views of
            # the ring slices), reduce each slice's 8 blocks on the DVE.
            for sl8 in range(8):
                for k in range(8):
                    blk = sl8 * 8 + k
                    nc.tensor.transpose(
                        out=pring_bf[:, sl8, k * P : (k + 1) * P],
                        in_=colacc[:, blk * P : (blk + 1) * P],
                        identity=eye,
                    )
                nc.vector.tensor_reduce(
                    out=c2[:, sl8 * 8 : (sl8 + 1) * 8],
                    in_=pring_bf[:, sl8, :].rearrange("p (b f) -> p b f", f=P),
                    axis=X,
                    op=Alu.min,
                )

            nc.vector.tensor_scalar_max(out=d1t, in0=c1, scalar1=0.0)
            nc.vector.tensor_scalar_max(out=d2t, in0=c2, scalar1=0.0)

        if repeat == 1:
            body()
        else:
            with tc.For_i(0, repeat, 1):
                body()

        nc.sync.dma_start(out=d1_d.ap(), in_=d1t)
        nc.sync.dma_start(out=d2_d.ap(), in_=d2t)

    nc.compile()
    return nc


_CACHED_NC = None


def _get_nc():
    global _CACHED_NC
    if _CACHED_NC is None:
        _CACHED_NC = build_bass()
    return _CACHED_NC


_EYE = np.eye(P, dtype=_BF16)


def _make_in_maps(xyz1, xyz2):
    xyz1 = np.asarray(xyz1, dtype=np.float32)
    xyz2 = np.asarray(xyz2, dtype=np.float32)
    in_maps = []
    for c in range(NCORES):
        b, h = divmod(c, 2)
        x = xyz1[b, h * NLOC : (h + 1) * NLOC]
        y = xyz2[b]
        xa, ya = _build_aug(x, y)
        in_maps.append({"xa": xa, "ya": ya, "eye": _EYE})
    return in_maps


def _unshard(results):
    dist1 = np.empty((B, N), np.float32)
    dist2 = np.empty((B, M), np.float32)
    for c in range(NCORES):
        b, h = divmod(c, 2)
        dist1[b, h * NLOC : (h + 1) * NLOC] = np.asarray(results[c]["d1"]).T.ravel()
        d2p = np.asarray(results[c]["d2"]).T.ravel()
        if h == 0:
            dist2[b] = d2p
        else:
            np.minimum(dist2[b], d2p, out=dist2[b])
    return dist1, dist2


def kernel(xyz1, xyz2):
    from concourse.bass_utils import run_bass_kernel_spmd

    nc = _get_nc()
    in_maps = _make_in_maps(xyz1, xyz2)
    res = run_bass_kernel_spmd(nc, in_maps, core_ids=list(range(NCORES)))
    return _unshard(res.results)
